# revision 2
# baseline (speedup 1.0000x reference)
"""TP=8 LSTM decoder kernel for trn2 (Bass, raw engine programming).

Math (per reference, with feedback folded into the recurrence):
    x(t) = y(t-1) = h(t-1) @ W_out.T + b_out   (x(0) = 0)
    gates(t) = x(t) @ W_ih.T + h(t-1) @ W_hh.T + b
             = h(t-1) @ W_comb.T + b'          (t >= 1)
    where W_comb = W_hh + W_ih @ W_out,  b' = b_ih + b_hh + W_ih @ b_out.
Step 0 (x=0) is computed on the host; the device runs steps 1..T-1 and
computes y(t) = h(t) @ W_out.T + b_out for t = 0..T-1.

Sharding: core s owns gate rows [g*2048 + s*256, +256) for each gate g
(i,f,g,o), i.e. 1024 of 8192 gate rows, and h-slice [s*256, +256).
Each step: gates matmul in [batch(64) x j(1024)] layout (activations
stationary, weights moving), LSTM cell on ACT/DVE, PE-transpose of the
h-slice, then an 8-way SBUF-to-SBUF remote-DMA broadcast (or ncfw
AllGather) to re-assemble h.T on every core.
"""

import numpy as np

B = 64          # batch
H = 2048        # lstm dim
MEL = 512
NC = 8
HS = H // NC    # 256 h rows per core
GS = 4 * HS     # 1024 gate rows per core
KCH = H // 128  # 16 contraction chunks


def build_nc(n_steps: int, comm: str = "rdma", debug: bool = False,
             coltile: bool = False, tiny_y: bool = False):
    import concourse.bass as bass
    import concourse.bacc as bacc
    import concourse.mybir as mybir
    from concourse.bass import ts

    f32 = mybir.dt.float32
    T = n_steps

    nc = bacc.Bacc("TRN2", target_bir_lowering=False, debug=debug,
                   num_devices=NC)

    # ---------------- I/O ----------------
    wct_d = nc.dram_tensor("wct", [128, KCH * GS], f32, kind="ExternalInput")
    wot_d = nc.dram_tensor("wot", [128, KCH * MEL], f32, kind="ExternalInput")
    h0t_d = nc.dram_tensor("h0t", [128, KCH * B], f32, kind="ExternalInput")
    c0s_d = nc.dram_tensor("c0s", [128, 128] if coltile else [B, HS], f32, kind="ExternalInput")
    bps_d = nc.dram_tensor("bps", [1, GS], f32, kind="ExternalInput")
    bout_d = nc.dram_tensor("bout", [1, MEL], f32, kind="ExternalInput")
    ones_d = nc.dram_tensor("ones", [1, B], f32, kind="ExternalInput")
    ident_d = nc.dram_tensor("ident", [128, B] if coltile else [B, B], f32, kind="ExternalInput")
    yout_d = nc.dram_tensor("yout", [1 if tiny_y else 512, B, MEL], f32, kind="ExternalOutput")

    if comm == "ncfw":
        cc_in = nc.dram_tensor("cc_in", [128, 128], f32)
        cc_out = [nc.dram_tensor(f"cc_out{b}", [NC * 128, 128], f32,
                                 addr_space="Shared") for b in range(2)]

    ctx_list = []

    def sb(name, shape):
        t = nc.sbuf_tensor(name, shape, f32)
        ctx_list.append(t)
        return t.__enter__()

    def ps(name, shape):
        t = nc.psum_tensor(name, shape, f32)
        ctx_list.append(t)
        return t.__enter__()

    def sem(name):
        t = nc.semaphore(name)
        ctx_list.append(t)
        return t.__enter__()

    # ---------------- SBUF ----------------
    s_wct = sb("s_wct", [128, KCH * GS])      # W_comb.T chunks (8 MB)
    s_wot = sb("s_wot", [128, KCH * MEL])     # W_out.T chunks (4 MB)
    s_hT = [sb("s_hT0", [128, KCH * B]), sb("s_hT1", [128, KCH * B])]
    if coltile:
        # [128, 128]: partitions 0:64 = batch x h-half0, 64:128 = batch x h-half1
        s_c = [sb("s_c0", [128, 128]), sb("s_c1", [128, 128])]
        s_sif = sb("s_sif", [128, 256])
        s_tg = sb("s_tg", [128, 128])
        s_so = sb("s_so", [128, 128])
        s_tc = sb("s_tc", [128, 128])
        s_t1 = sb("s_t1", [128, 128])
        s_t2 = sb("s_t2", [128, 128])
        s_h = sb("s_h", [128, 128])
    else:
        s_c = [sb("s_c0", [B, HS]), sb("s_c1", [B, HS])]
        s_sif = sb("s_sif", [B, 2 * HS])
        s_tg = sb("s_tg", [B, HS])
        s_so = sb("s_so", [B, HS])
        s_tc = sb("s_tc", [B, HS])
        s_t1 = sb("s_t1", [B, HS])
        s_t2 = sb("s_t2", [B, HS])
        s_h = sb("s_h", [B, HS])
    s_stage = sb("s_stage", [128, 128])
    s_y = [sb("s_y0", [B, MEL]), sb("s_y1", [B, MEL])]
    s_bps = sb("s_bps", [1, GS])
    s_bout = sb("s_bout", [1, MEL])
    s_ones = sb("s_ones", [1, B])
    s_ident = sb("s_ident", [128, B] if coltile else [B, B])

    # ---------------- PSUM ----------------
    if coltile:
        p_g = ps("p_g", [128, 512])   # partitions 0:64 = tile0, 64:128 = tile1
    else:
        p_g0 = ps("p_g0", [B, 512])    # gates j 0:512  (i|f)
        p_g1 = ps("p_g1", [B, 512])    # gates j 512:1024 (g|o)
    p_y = ps("p_y", [B, MEL])
    p_tr = ps("p_tr", [128, 128])

    # ---------------- semaphores ----------------
    s_pre = sem("s_pre")      # preload DMAs
    s_recv = sem("s_recv")    # remote arrivals (16/round)
    s_send = sem("s_send")    # local broadcast completion (16/round)
    s_gates = sem("s_gates")  # PE: gates done (t)
    s_act1 = sem("s_act1")    # ACT: sif+tg done (t)
    s_dvec = sem("s_dvec")    # DVE: c_new done (t)
    s_act2 = sem("s_act2")    # ACT: tanh(c) done (t)
    s_dveh = sem("s_dveh")    # DVE: h done (t)
    s_tr = sem("s_tr")        # PE: transposes done (t)
    s_stg = sem("s_stg")      # DVE: staging copy done (t)
    s_yv = sem("s_yv")        # PE: y(t-1) matmuls done (value t)
    s_ycp = sem("s_ycp")      # DVE: y psum copy done (value t)
    s_ydma = sem("s_ydma")    # sync: y store done (16 per y)
    if comm == "ncfw":
        s_ccin = sem("s_ccin")    # staging -> DRAM done (16/round)
        s_cc = sem("s_cc")        # collective done (1/round)
        s_hin = sem("s_hin")      # cc_out -> SBUF done (16*4/round)

    N_PRE = 8  # preload DMA count (wct, wot, h0t, c0s, bps, bout, ones, ident)

    with nc.Block() as block:

        # ------------- sync engine: preloads + y stores -------------
        @block.sync
        def _(sync):
            sync.dma_start(out=s_wct[:, :], in_=wct_d[:, :]).then_inc(s_pre, 16)
            sync.dma_start(out=s_hT[0][:, :], in_=h0t_d[:, :]).then_inc(s_pre, 16)
            sync.dma_start(out=s_c[0][:, :], in_=c0s_d[:, :]).then_inc(s_pre, 16)
            sync.dma_start(out=s_bps[:, :], in_=bps_d[:, :]).then_inc(s_pre, 16)
            sync.dma_start(out=s_bout[:, :], in_=bout_d[:, :]).then_inc(s_pre, 16)
            sync.dma_start(out=s_ones[:, :], in_=ones_d[:, :]).then_inc(s_pre, 16)
            sync.dma_start(out=s_ident[:, :], in_=ident_d[:, :]).then_inc(s_pre, 16)
            sync.dma_start(out=s_wot[:, :], in_=wot_d[:, :]).then_inc(s_pre, 16)
            for t in range(1, T + 1):
                if comm == "ncfw" and t <= T - 1:
                    # scatter cc_out blocks into hT buffer columns
                    sync.wait_ge(s_cc, t)
                    if t >= 2:
                        sync.wait_ge(s_hin, 16 * (t - 1))
                    sync.dma_start(
                        out=s_hT[t % 2][:, :].rearrange("p (r c) -> p r c", r=NC),
                        in_=cc_out[t % 2][:, :].rearrange("(r p) c -> p r c", r=NC),
                    ).then_inc(s_hin, 16)
                # store y(t-1); serialize sem updates (order across HW queues)
                sync.wait_ge(s_ycp, t)
                if t >= 2:
                    sync.wait_ge(s_ydma, 16 * (t - 1))
                sync.dma_start(
                    out=yout_d[0 if tiny_y else t - 1, :, :],
                    in_=s_y[(t - 1) % 2][:, :]
                ).then_inc(s_ydma, 16)

        # ------------- PE -------------
        @block.tensor
        def _(pe):
            pe.wait_ge(s_pre, 16 * N_PRE)
            for t in range(1, T + 1):
                rb = (t - 1) % 2          # h(t-1) buffer
                hbuf = s_hT[rb]
                if t >= 2:
                    if comm == "rdma":
                        pe.wait_ge(s_recv, 16 * (t - 1))
                    else:
                        pe.wait_ge(s_hin, 16 * (t - 1))
                    pe.wait_ge(s_stg, t - 1)   # p_tr WAR
                    pe.wait_ge(s_ycp, t - 1)   # p_y WAR
                if t <= T - 1:
                    # gates(t) = h(t-1) @ Wcomb.T + b'
                    if coltile:
                        nc.tensor.matmul(p_g[0:64, :], s_ones[:, :],
                                         s_bps[:, 0:512], start=True, stop=False,
                                         tile_position=(0, 0))
                        for k in range(KCH):
                            nc.tensor.matmul(p_g[0:64, :], hbuf[:, ts(k, B)],
                                             s_wct[:, k * GS: k * GS + 512],
                                             start=False, stop=(k == KCH - 1),
                                             tile_position=(0, 0))
                        nc.tensor.matmul(p_g[64:128, :], s_ones[:, :],
                                         s_bps[:, 512:1024], start=True, stop=False,
                                         tile_position=(0, 64))
                        for k in range(KCH):
                            mm = nc.tensor.matmul(p_g[64:128, :], hbuf[:, ts(k, B)],
                                                  s_wct[:, k * GS + 512: (k + 1) * GS],
                                                  start=False, stop=(k == KCH - 1),
                                                  tile_position=(0, 64))
                    else:
                        nc.tensor.matmul(p_g0[:, :], s_ones[:, :],
                                         s_bps[:, 0:512], start=True, stop=False)
                        nc.tensor.matmul(p_g1[:, :], s_ones[:, :],
                                         s_bps[:, 512:1024], start=True, stop=False)
                        for k in range(KCH):
                            lhsT = hbuf[:, ts(k, B)]
                            last = k == KCH - 1
                            nc.tensor.matmul(p_g0[:, :], lhsT,
                                             s_wct[:, k * GS: k * GS + 512],
                                             start=False, stop=last)
                            mm = nc.tensor.matmul(p_g1[:, :], lhsT,
                                                  s_wct[:, k * GS + 512: (k + 1) * GS],
                                                  start=False, stop=last)
                    mm.then_inc(s_gates, 1)
                # y(t-1) = h(t-1) @ Wout.T + b_out
                nc.tensor.matmul(p_y[:, :], s_ones[:, :], s_bout[:, :],
                                 start=True, stop=False)
                for k in range(KCH):
                    mm = nc.tensor.matmul(p_y[:, :], hbuf[:, ts(k, B)],
                                          s_wot[:, ts(k, MEL)],
                                          start=False, stop=(k == KCH - 1))
                mm.then_inc(s_yv, 1)
                if t <= T - 1:
                    # transpose h slice -> p_tr
                    pe.wait_ge(s_dveh, t)
                    if coltile:
                        nc.tensor.transpose(p_tr[:, 0:B], s_h[0:64, :],
                                            s_ident[0:64, :])
                        nc.tensor.transpose(p_tr[:, B:128], s_h[64:128, :],
                                            s_ident[64:128, :]).then_inc(s_tr, 1)
                    else:
                        nc.tensor.transpose(p_tr[:, 0:B], s_h[:, 0:128],
                                            s_ident[:, :])
                        nc.tensor.transpose(p_tr[:, B:128], s_h[:, 128:256],
                                            s_ident[:, :]).then_inc(s_tr, 1)

        # ------------- ACT -------------
        @block.scalar
        def _(act):
            act.wait_ge(s_pre, 16 * N_PRE)
            Sig = mybir.ActivationFunctionType.Sigmoid
            Tanh = mybir.ActivationFunctionType.Tanh
            for t in range(1, T):
                act.wait_ge(s_gates, t)
                if coltile:
                    nc.scalar.activation(s_sif[:, :], p_g[:, 0:256], Sig)
                    nc.scalar.activation(s_tg[:, :], p_g[:, 256:384], Tanh)\
                        .then_inc(s_act1, 1)
                    nc.scalar.activation(s_so[:, :], p_g[:, 384:512], Sig)
                else:
                    nc.scalar.activation(s_sif[:, :], p_g0[:, :], Sig)
                    nc.scalar.activation(s_tg[:, :], p_g1[:, 0:HS], Tanh)\
                        .then_inc(s_act1, 1)
                    nc.scalar.activation(s_so[:, :], p_g1[:, HS:2 * HS], Sig)
                act.wait_ge(s_dvec, t)
                nc.scalar.activation(s_tc[:, :], s_c[t % 2][:, :], Tanh)\
                    .then_inc(s_act2, 1)
                if comm == "ncfw":
                    act.wait_ge(s_stg, t)
                    if t >= 2:
                        act.wait_ge(s_ccin, 16 * (t - 1))
                    act.dma_start(out=cc_in[:, :], in_=s_stage[:, :])\
                        .then_inc(s_ccin, 16)

        # ------------- DVE -------------
        @block.vector
        def _(dve):
            dve.wait_ge(s_pre, 16 * N_PRE)
            mult = mybir.AluOpType.mult
            add = mybir.AluOpType.add
            for t in range(1, T + 1):
                if t <= T - 1:
                    HW = 128 if coltile else HS
                    dve.wait_ge(s_act1, t)
                    nc.vector.scalar_tensor_tensor(
                        s_t1[:, :], s_sif[:, 0:HW], 1.0, s_tg[:, :], mult, mult)
                    nc.vector.scalar_tensor_tensor(
                        s_t2[:, :], s_sif[:, HW:2 * HW], 1.0,
                        s_c[(t - 1) % 2][:, :], mult, mult)
                    dve.drain()
                    nc.vector.scalar_tensor_tensor(
                        s_c[t % 2][:, :], s_t1[:, :], 1.0, s_t2[:, :],
                        mult, add).then_inc(s_dvec, 1)
                    dve.drain()
                    dve.wait_ge(s_act2, t)
                    nc.vector.scalar_tensor_tensor(
                        s_h[:, :], s_so[:, :], 1.0, s_tc[:, :], mult, mult)\
                        .then_inc(s_dveh, 1)
                # y(t-1) psum -> sbuf  (after PE y matmuls)
                dve.wait_ge(s_yv, t)
                if t >= 3:
                    dve.wait_ge(s_ydma, 16 * (t - 2))  # s_y buf free
                nc.vector.tensor_copy(s_y[(t - 1) % 2][:, :], p_y[:, :])\
                    .then_inc(s_ycp, 1)
                if t <= T - 1:
                    # staging copy (after PE transposes)
                    dve.wait_ge(s_tr, t)
                    if comm == "rdma" and t >= 2:
                        dve.wait_ge(s_send, 16 * (t - 1))  # staging free
                    if comm == "ncfw" and t >= 2:
                        dve.wait_ge(s_ccin, 16 * (t - 1))
                    nc.vector.tensor_copy(s_stage[:, :], p_tr[:, :])\
                        .then_inc(s_stg, 1)

        # ------------- gpsimd: comm -------------
        @block.gpsimd
        def _(gpsimd):
            gpsimd.wait_ge(s_pre, 16 * N_PRE)
            if comm == "rdma":
                pid = gpsimd.partition_id()
                for t in range(1, T):
                    gpsimd.wait_ge(s_stg, t)
                    gpsimd.remote_dma_broadcast(
                        s_hT[t % 2][:, ts(pid, 128)],
                        s_stage[:, :],
                        remote_sem=s_recv,
                        local_sem=s_send,
                        rdests=[(0, k) for k in range(NC)],
                    )
                    gpsimd.trigger_dma(count=1)
            else:
                for t in range(1, T):
                    gpsimd.wait_ge(s_ccin, 16 * t)
                    gpsimd.collective_compute(
                        "AllGather",
                        mybir.AluOpType.bypass,
                        replica_groups=[list(range(NC))],
                        ins=[cc_in.ap().opt()],
                        outs=[cc_out[t % 2].ap().opt()],
                    ).then_inc(s_cc)

    for c in reversed(ctx_list):
        c.__exit__(None, None, None)

    nc.compile()
    return nc


# ---------------------------------------------------------------------------
# host side
# ---------------------------------------------------------------------------

def _sigmoid(x):
    return 1.0 / (1.0 + np.exp(-x))


def prepare_inputs(inputs: dict, n_steps: int, coltile: bool = False):
    """Host-side fold + step 0; returns per-core in_maps."""
    h0 = np.asarray(inputs["h0"])[0].astype(np.float32)      # [B, H]
    c0 = np.asarray(inputs["c0"])[0].astype(np.float32)
    W_ih = np.asarray(inputs["W_ih"]).astype(np.float32)     # [4H, 512]
    W_hh = np.asarray(inputs["W_hh"]).astype(np.float32)     # [4H, H]
    b = (np.asarray(inputs["b_ih"]) + np.asarray(inputs["b_hh"])).astype(np.float32)
    W_out = np.asarray(inputs["W_out"]).astype(np.float32)   # [MEL, H]
    b_out = np.asarray(inputs["b_out"]).astype(np.float32)

    W_comb = W_hh + W_ih @ W_out                             # [4H, H]
    bp = b + W_ih @ b_out                                    # [4H]

    # host step 0 (x = 0)
    gates0 = h0 @ W_hh.T + b
    i0, f0, g0, o0 = np.split(gates0, 4, axis=1)
    c1 = _sigmoid(f0) * c0 + _sigmoid(i0) * np.tanh(g0)
    h1 = _sigmoid(o0) * np.tanh(c1)                          # h(0) [B, H]

    hT = np.ascontiguousarray(h1.T)                          # [H, B]
    h0t = hT.reshape(KCH, 128, B).transpose(1, 0, 2).reshape(128, KCH * B)
    # layout check: h0t[:, 64c:64c+64] == hT[128c:128c+128]
    WoutT = np.ascontiguousarray(W_out.T)                    # [H, MEL]
    wot = WoutT.reshape(KCH, 128, MEL).transpose(1, 0, 2).reshape(128, KCH * MEL)

    in_maps = []
    for s in range(NC):
        if coltile:
            # j order: [half th=0: i,f,g,o (128 each) | half th=1: i,f,g,o]
            rows = np.concatenate(
                [np.arange(g * H + s * HS + th * 128,
                           g * H + s * HS + th * 128 + 128)
                 for th in range(2) for g in range(4)])
            cs = c1[:, s * HS:(s + 1) * HS]                  # [B, 256]
            c0s = np.concatenate([cs[:, 0:128], cs[:, 128:256]], axis=0)
            ident = np.concatenate([np.eye(B, dtype=np.float32)] * 2, axis=0)
        else:
            rows = np.concatenate(
                [np.arange(g * H + s * HS, g * H + (s + 1) * HS)
                 for g in range(4)])
            c0s = c1[:, s * HS:(s + 1) * HS]
            ident = np.eye(B, dtype=np.float32)
        WcT = np.ascontiguousarray(W_comb[rows, :].T)        # [H, GS]
        wct = WcT.reshape(KCH, 128, GS).transpose(1, 0, 2).reshape(128, KCH * GS)
        in_maps.append({
            "wct": np.ascontiguousarray(wct),
            "wot": np.ascontiguousarray(wot),
            "h0t": np.ascontiguousarray(h0t),
            "c0s": np.ascontiguousarray(c0s),
            "bps": np.ascontiguousarray(bp[rows][None, :]),
            "bout": np.ascontiguousarray(b_out[None, :]),
            "ones": np.ones((1, B), np.float32),
            "ident": np.ascontiguousarray(ident),
        })
    return in_maps


# ---------------------------------------------------------------------------
# harness entry point
# ---------------------------------------------------------------------------

def run(inputs, trace=False):
    T = 512
    nc = build_nc(T, comm="ncfw", debug=False, coltile=False)
    in_maps = prepare_inputs(inputs, T, coltile=False)
    from concourse import bass_utils
    return bass_utils.run_bass_kernel_spmd(
        nc, in_maps, core_ids=list(range(NC)), trace=trace)


def assemble_output(res):
    y = res.results[0]["yout"][:512]        # [T, B, MEL]
    return np.ascontiguousarray(np.transpose(y, (1, 0, 2)).astype(np.float32))


def kernel(**inputs):
    """Full-input/full-output entry. Distributes across 8 NeuronCores (TP over
    the 4H gate dim) internally; returns y [B, T, MEL] float32."""
    return assemble_output(run(inputs))



# revision 19
# speedup vs baseline: 3.7842x; 3.7842x over previous
"""TP=8 LSTM decoder kernel for trn2 (Bass, raw engine programming), v2.

Math (per reference, with feedback folded into the recurrence):
    x(t) = y(t-1) = h(t-1) @ W_out.T + b_out   (x(0) = 0)
    gates(t) = x(t) @ W_ih.T + h(t-1) @ W_hh.T + b
             = h(t-1) @ W_comb.T + b'          (t >= 1)
    where W_comb = W_hh + W_ih @ W_out,  b' = b_ih + b_hh + W_ih @ b_out.
Step 0 (x=0) is computed on the host; the device runs steps 1..T-1 and
computes y(t) = h(t) @ W_out.T + b_out for t = 0..T-1.

v2 vs v1:
  - all matmul operands fp16 (1 cycle/row on PE vs fp32's 4), fp32 PSUM
  - y output TP-sliced 8 ways (each core computes mel rows [64s, 64s+64));
    host assembles from all 8 cores' DRAM outputs
  - per-core gate rows split in two 128-row blocks (r0/r1); block r0's
    cell + transpose + broadcast overlap the PE phase for block r1, and
    block r1's tail overlaps the next step's r0 phase
  - h exchanged via SBUF-to-SBUF remote_dma_broadcast (16 KB per block)
    instead of an AllGather bounced through DRAM
  - receive semaphores split by step parity to close the 1-step-skew
    anonymous-counter race

Sharding: core s owns h rows [256s, 256s+256) as two blocks of 128
(gate col order [i|f|o|g] per block) and mel rows [64s, 64s+64).
"""

import numpy as np

B = 64          # batch
H = 2048        # lstm dim
MEL = 512
NC = 8
HS = H // NC    # 256 h rows per core
KCH = H // 128  # 16 contraction chunks
T = 512


def build_nc(n_steps: int, debug: bool = False):
    import concourse.bass as bass
    import concourse.bacc as bacc
    import concourse.mybir as mybir
    from concourse.bass import ts, ds

    f32 = mybir.dt.float32
    f16 = mybir.dt.float16
    Tn = n_steps

    nc = bacc.Bacc("TRN2", target_bir_lowering=False, debug=debug,
                   num_devices=NC)

    # ---------------- DRAM I/O ----------------
    w_d = nc.dram_tensor("w", [128, KCH * 1024], f16, kind="ExternalInput")
    wy_d = nc.dram_tensor("wy", [128, KCH * 64], f16, kind="ExternalInput")
    h0_d = nc.dram_tensor("h0", [128, KCH * 64], f16, kind="ExternalInput")
    c0_d = nc.dram_tensor("c0", [B, 2 * 128], f32, kind="ExternalInput")
    bps_d = nc.dram_tensor("bps", [1, 1024], f16, kind="ExternalInput")
    bout_d = nc.dram_tensor("bout", [1, 64], f16, kind="ExternalInput")
    ones_d = nc.dram_tensor("ones", [1, B], f16, kind="ExternalInput")
    id_d = nc.dram_tensor("ident", [B, B], f32, kind="ExternalInput")
    yout_d = nc.dram_tensor("yout", [Tn, B, 64], f32, kind="ExternalOutput")

    ctx_list = []

    def sb(name, shape, dt=f32):
        t = nc.sbuf_tensor(name, shape, dt)
        ctx_list.append(t)
        return t.__enter__()

    def ps(name, shape):
        t = nc.psum_tensor(name, shape, f32)
        ctx_list.append(t)
        return t.__enter__()

    def sem(name):
        t = nc.semaphore(name)
        ctx_list.append(t)
        return t.__enter__()

    # ---------------- SBUF ----------------
    s_w = sb("s_w", [128, KCH * 1024], f16)    # 4 MB: chunk c -> [r0 512|r1 512]
    s_wy = sb("s_wy", [128, KCH * 64], f16)    # 256 KB
    s_h = [sb("s_h0", [128, KCH * 64], f16),   # h.T chunks, dbl-buffered
           sb("s_h1", [128, KCH * 64], f16)]
    s_bps = sb("s_bps", [1, 1024], f16)
    s_bout = sb("s_bout", [1, 64], f16)
    s_ones = sb("s_ones", [1, B], f16)
    s_id = sb("s_id", [B, B])
    s_c = sb("s_c", [B, 2 * 256])              # c state, parity cols [r0|r1]
    s_act = sb("s_act", [B, 2 * 512])          # per block: sig(i|f|o) 384 | tg 128
    s_tc = sb("s_tc", [B, 2 * 128])            # tanh(c) per block
    s_t1a = sb("s_t1a", [B, 128])
    s_t2a = sb("s_t2a", [B, 128])
    s_t1b = sb("s_t1b", [B, 128])
    s_t2b = sb("s_t2b", [B, 128])
    s_hraw = sb("s_hraw", [B, 256])            # h fp32 [r0 128 | r1 128]
    s_stage = sb("s_stage", [128, 2 * 128], f16)  # h.T bcast staging, parity
    s_y = sb("s_y", [B, 2 * 64])               # y slice staging, parity

    # ---------------- PSUM (one bank each) ----------------
    p_g0 = ps("p_g0", [B, 512])     # block r0 gates [i|f|o|g] x128
    p_g1 = ps("p_g1", [B, 512])     # block r1 gates
    p_y = ps("p_y", [B, 64])        # y slice [batch, mel-slice]
    p_tr0 = ps("p_tr0", [128, 64])  # h.T block r0
    p_tr1 = ps("p_tr1", [128, 64])  # h.T block r1

    # ---------------- semaphores ----------------
    s_pre = sem("s_pre")        # preload DMAs (16 each)
    s_g0 = sem("s_g0")          # PE: p_g0 done (1/step)
    s_g1 = sem("s_g1")          # PE: p_g1 done (1/step)
    s_a0 = sem("s_a0")          # ACT: sig/tanh r0 done (1/step)
    s_a1 = sem("s_a1")          # ACT: sig/tanh r1 done (1/step)
    s_cd = sem("s_cd")          # DVE: c_new done (2/step)
    s_tcs = sem("s_tcs")        # ACT: tanh(c) done (2/step)
    s_hd = sem("s_hd")          # DVE: h fp32 done (2/step)
    s_tr = sem("s_tr")          # PE: transpose done (2/step)
    s_hl = sem("s_hl")          # DVE: stage copy into own h.T slot (2/step)
    s_hr0 = [sem("s_hr0e"), sem("s_hr0o")]   # remote r0 arrivals, by parity
    s_hr1 = [sem("s_hr1e"), sem("s_hr1o")]   # remote r1 arrivals, by parity
    s_send = sem("s_send")      # broadcast local completion (16/bcast)
    s_prep = sem("s_prep")      # broadcast descriptor-write completion
    s_yv = sem("s_yv")          # PE: y matmuls done (1/step)
    s_ycp = sem("s_ycp")        # DVE: y psum->sbuf copy done (1/step)
    s_ydma = sem("s_ydma")      # sync: y store done (16/step)

    N_PRE = 8
    mult = mybir.AluOpType.mult
    add = mybir.AluOpType.add
    Sig = mybir.ActivationFunctionType.Sigmoid
    Tanh = mybir.ActivationFunctionType.Tanh

    # number of steps t' in [1, t] with parity t'%2 == p
    def npar(t, p):
        if t <= 0:
            return 0
        return (t + 1) // 2 if p == 1 else t // 2

    with nc.Block() as block:

        # ------------- sync: preloads + y stores -------------
        @block.sync
        def _(sync):
            sync.dma_start(out=s_w[:, :], in_=w_d[:, :]).then_inc(s_pre, 16)
            sync.dma_start(out=s_wy[:, :], in_=wy_d[:, :]).then_inc(s_pre, 16)
            sync.dma_start(out=s_h[0][:, :], in_=h0_d[:, :]).then_inc(s_pre, 16)
            sync.dma_start(out=s_c[:, 0:256], in_=c0_d[:, :]).then_inc(s_pre, 16)
            sync.dma_start(out=s_bps[:, :], in_=bps_d[:, :]).then_inc(s_pre, 16)
            sync.dma_start(out=s_bout[:, :], in_=bout_d[:, :]).then_inc(s_pre, 16)
            sync.dma_start(out=s_ones[:, :], in_=ones_d[:, :]).then_inc(s_pre, 16)
            sync.dma_start(out=s_id[:, :], in_=id_d[:, :]).then_inc(s_pre, 16)
            for t in range(1, Tn + 1):
                sync.wait_ge(s_ycp, t)
                if t >= 2:
                    sync.wait_ge(s_ydma, 16 * (t - 1))
                sync.dma_start(
                    out=yout_d[t - 1, :, :],
                    in_=s_y[:, ts((t - 1) % 2, 64)],
                ).then_inc(s_ydma, 16)

        # ------------- PE -------------
        @block.tensor
        def _(pe):
            pe.wait_ge(s_pre, 16 * N_PRE)
            for t in range(1, Tn + 1):
                hb = s_h[(t - 1) % 2]
                par = (t - 1) % 2
                if t <= Tn - 1:
                    # ---- phase 1: block r0 gates into p_g0 ----
                    if t >= 2:
                        pe.wait_ge(s_a0, t - 1)          # p_g0 free
                        pe.wait_ge(s_hr0[par], 16 * npar(t - 1, par))
                    nc.tensor.matmul(p_g0[:, :], s_ones[:, :], s_bps[:, 0:512],
                                     start=True, stop=False)
                    for c in range(0, KCH, 2):           # even chunks (r0 of peers)
                        nc.tensor.matmul(p_g0[:, :], hb[:, ts(c, 64)],
                                         s_w[:, c * 1024: c * 1024 + 512],
                                         start=False, stop=False)
                    if t >= 2:
                        pe.wait_ge(s_hr1[par], 16 * npar(t - 1, par))
                    for c in range(1, KCH, 2):           # odd chunks (r1 of peers)
                        mm = nc.tensor.matmul(p_g0[:, :], hb[:, ts(c, 64)],
                                              s_w[:, c * 1024: c * 1024 + 512],
                                              start=False, stop=(c == KCH - 1))
                    mm.then_inc(s_g0, 1)
                    # ---- phase 2: block r1 gates into p_g1 ----
                    if t >= 2:
                        pe.wait_ge(s_a1, t - 1)          # p_g1 free
                    nc.tensor.matmul(p_g1[:, :], s_ones[:, :], s_bps[:, 512:1024],
                                     start=True, stop=False)
                    for c in range(0, KCH, 2):
                        nc.tensor.matmul(p_g1[:, :], hb[:, ts(c, 64)],
                                         s_w[:, c * 1024 + 512: (c + 1) * 1024],
                                         start=False, stop=False)
                    # transpose r0 mid-phase2 (cell r0 ran during the above)
                    pe.wait_ge(s_hd, 2 * t - 1)
                    if t >= 2:
                        pe.wait_ge(s_hl, 2 * (t - 1) - 1)   # p_tr0 free
                    nc.tensor.transpose(p_tr0[:, :], s_hraw[:, 0:128],
                                        s_id[:, :]).then_inc(s_tr, 1)
                    for c in range(1, KCH, 2):
                        mm = nc.tensor.matmul(p_g1[:, :], hb[:, ts(c, 64)],
                                              s_w[:, c * 1024 + 512: (c + 1) * 1024],
                                              start=False, stop=(c == KCH - 1))
                    mm.then_inc(s_g1, 1)
                # ---- y(t-1) slice ----
                pe.wait_ge(s_ycp, t - 1)                 # p_y free
                nc.tensor.matmul(p_y[:, :], s_ones[:, :], s_bout[:, :],
                                 start=True, stop=False)
                for c in range(KCH):
                    mm = nc.tensor.matmul(p_y[:, :], hb[:, ts(c, 64)],
                                          s_wy[:, ts(c, 64)],
                                          start=False, stop=(c == KCH - 1))
                mm.then_inc(s_yv, 1)
                if t <= Tn - 1:
                    # transpose r1 (tail; overlaps next step's phase 1)
                    pe.wait_ge(s_hd, 2 * t)
                    if t >= 2:
                        pe.wait_ge(s_hl, 2 * (t - 1))    # p_tr1 free
                    nc.tensor.transpose(p_tr1[:, :], s_hraw[:, 128:256],
                                        s_id[:, :]).then_inc(s_tr, 1)

        # ------------- ACT -------------
        @block.scalar
        def _(act):
            act.wait_ge(s_pre, 16 * N_PRE)
            for t in range(1, Tn):
                # block r0
                act.wait_ge(s_g0, t)
                if t >= 2:
                    act.wait_ge(s_hd, 2 * (t - 1) - 1)   # s_act r0 free
                nc.scalar.activation(s_act[:, 0:384], p_g0[:, 0:384], Sig)
                nc.scalar.activation(s_act[:, 384:512], p_g0[:, 384:512], Tanh)\
                    .then_inc(s_a0, 1)
                act.wait_ge(s_cd, 2 * t - 1)
                nc.scalar.activation(s_tc[:, 0:128], s_c[:, ts(t % 2, 256)][:, 0:128],
                                     Tanh).then_inc(s_tcs, 1)
                # block r1
                act.wait_ge(s_g1, t)
                if t >= 2:
                    act.wait_ge(s_hd, 2 * (t - 1))       # s_act r1 free
                nc.scalar.activation(s_act[:, 512:896], p_g1[:, 0:384], Sig)
                nc.scalar.activation(s_act[:, 896:1024], p_g1[:, 384:512], Tanh)\
                    .then_inc(s_a1, 1)
                act.wait_ge(s_cd, 2 * t)
                nc.scalar.activation(s_tc[:, 128:256],
                                     s_c[:, ts(t % 2, 256)][:, 128:256],
                                     Tanh).then_inc(s_tcs, 1)

        # ------------- DVE -------------
        @block.vector
        def _(dve):
            dve.wait_ge(s_pre, 16 * N_PRE)
            for t in range(1, Tn + 1):
                cold = s_c[:, ts((t - 1) % 2, 256)]
                cnew = s_c[:, ts(t % 2, 256)]
                if t <= Tn - 1:
                    stg = s_stage[:, ts(t % 2, 128)]
                    # ---- block r0: c, h, stage (bcast can start mid-step) ----
                    dve.wait_ge(s_a0, t)
                    nc.vector.scalar_tensor_tensor(
                        s_t1a[:, :], s_act[:, 0:128], 1.0, s_act[:, 384:512],
                        mult, mult)
                    nc.vector.scalar_tensor_tensor(
                        s_t2a[:, :], s_act[:, 128:256], 1.0, cold[:, 0:128],
                        mult, mult)
                    dve.drain()
                    nc.vector.scalar_tensor_tensor(
                        cnew[:, 0:128], s_t1a[:, :], 1.0, s_t2a[:, :],
                        mult, add).then_inc(s_cd, 1)
                    dve.wait_ge(s_tcs, 2 * t - 1)
                    nc.vector.scalar_tensor_tensor(
                        s_hraw[:, 0:128], s_act[:, 256:384], 1.0, s_tc[:, 0:128],
                        mult, mult).then_inc(s_hd, 1)
                    dve.wait_ge(s_tr, 2 * t - 1)
                    if t >= 3:
                        dve.wait_ge(s_send, 32 * (t - 2))
                    nc.vector.tensor_copy(stg[:, 0:64], p_tr0[:, :])\
                        .then_inc(s_hl, 1)
                    # ---- block r1: c, h, stage ----
                    dve.wait_ge(s_a1, t)
                    nc.vector.scalar_tensor_tensor(
                        s_t1b[:, :], s_act[:, 512:640], 1.0, s_act[:, 896:1024],
                        mult, mult)
                    nc.vector.scalar_tensor_tensor(
                        s_t2b[:, :], s_act[:, 640:768], 1.0, cold[:, 128:256],
                        mult, mult)
                    dve.drain()
                    nc.vector.scalar_tensor_tensor(
                        cnew[:, 128:256], s_t1b[:, :], 1.0, s_t2b[:, :],
                        mult, add).then_inc(s_cd, 1)
                    dve.wait_ge(s_tcs, 2 * t)
                    nc.vector.scalar_tensor_tensor(
                        s_hraw[:, 128:256], s_act[:, 768:896], 1.0,
                        s_tc[:, 128:256], mult, mult).then_inc(s_hd, 1)
                    dve.wait_ge(s_tr, 2 * t)
                    nc.vector.tensor_copy(stg[:, 64:128], p_tr1[:, :])\
                        .then_inc(s_hl, 1)
                # y copy
                dve.wait_ge(s_yv, t)
                if t >= 3:
                    dve.wait_ge(s_ydma, 16 * (t - 2))
                nc.vector.tensor_copy(s_y[:, ts((t - 1) % 2, 64)], p_y[:, :])\
                    .then_inc(s_ycp, 1)

        # ------------- gpsimd: h broadcast -------------
        @block.gpsimd
        def _(gpsimd):
            gpsimd.wait_ge(s_pre, 16 * N_PRE)
            pid = gpsimd.partition_id()
            rd = [(0, k) for k in range(NC)]    # all 8 peers incl. self
            for t in range(1, Tn):
                hn = s_h[t % 2]
                stg = s_stage[:, ts(t % 2, 128)]
                gpsimd.wait_ge(s_hl, 2 * t - 1)
                gpsimd.wait_ge(s_yv, t - 1)   # old-parity h reads done (local)
                if t >= 2:
                    gpsimd.wait_ge(s_send, 32 * (t - 1))
                gpsimd.remote_dma_broadcast(
                    hn[:, ds(pid * 128, 64)], stg[:, 0:64],
                    remote_sem=s_hr0[t % 2], local_sem=s_send, rdests=rd)\
                    .then_inc(s_prep, 1)
                gpsimd.wait_ge(s_prep, 2 * t - 1)
                gpsimd.trigger_dma(count=1)
                gpsimd.wait_ge(s_hl, 2 * t)
                gpsimd.remote_dma_broadcast(
                    hn[:, ds(pid * 128 + 64, 64)], stg[:, 64:128],
                    remote_sem=s_hr1[t % 2], local_sem=s_send, rdests=rd)\
                    .then_inc(s_prep, 1)
                gpsimd.wait_ge(s_prep, 2 * t)
                gpsimd.trigger_dma(count=1)

    for c in reversed(ctx_list):
        c.__exit__(None, None, None)

    nc.compile()
    return nc


# ---------------------------------------------------------------------------
# host side
# ---------------------------------------------------------------------------

def _sigmoid(x):
    return 1.0 / (1.0 + np.exp(-x))


def prepare_inputs(inputs: dict):
    """Host-side fold + step 0; returns per-core in_maps."""
    h0 = np.asarray(inputs["h0"])[0].astype(np.float64)      # [B, H]
    c0 = np.asarray(inputs["c0"])[0].astype(np.float64)
    W_ih = np.asarray(inputs["W_ih"]).astype(np.float64)     # [4H, 512]
    W_hh = np.asarray(inputs["W_hh"]).astype(np.float64)     # [4H, H]
    b = (np.asarray(inputs["b_ih"]) + np.asarray(inputs["b_hh"])).astype(np.float64)
    W_out = np.asarray(inputs["W_out"]).astype(np.float64)   # [MEL, H]
    b_out = np.asarray(inputs["b_out"]).astype(np.float64)

    W_comb = W_hh + W_ih @ W_out                             # [4H, H]
    bp = b + W_ih @ b_out                                    # [4H]

    # host step 0 (x = 0)
    gates0 = h0 @ W_hh.T + b
    i0, f0, g0, o0 = np.split(gates0, 4, axis=1)
    c1 = _sigmoid(f0) * c0 + _sigmoid(i0) * np.tanh(g0)
    h1 = _sigmoid(o0) * np.tanh(c1)                          # h(0) [B, H]

    hT = np.ascontiguousarray(h1.T)                          # [H, B]
    h0t = hT.reshape(KCH, 128, B).transpose(1, 0, 2)\
        .reshape(128, KCH * B).astype(np.float16)

    in_maps = []
    GATE_ORDER = (0, 1, 3, 2)                                # [i|f|o|g]
    for s in range(NC):
        rows = np.concatenate(
            [np.arange(g * H + s * HS + r * 128, g * H + s * HS + r * 128 + 128)
             for r in range(2) for g in GATE_ORDER])         # [r0 512 | r1 512]
        WT = np.ascontiguousarray(W_comb[rows, :].T)         # [H, 1024]
        w = WT.reshape(KCH, 128, 1024).transpose(1, 0, 2)\
            .reshape(128, KCH * 1024).astype(np.float16)
        WyT = np.ascontiguousarray(W_out[s * 64:(s + 1) * 64, :].T)  # [H, 64]
        wy = WyT.reshape(KCH, 128, 64).transpose(1, 0, 2)\
            .reshape(128, KCH * 64).astype(np.float16)
        in_maps.append({
            "w": np.ascontiguousarray(w),
            "wy": np.ascontiguousarray(wy),
            "h0": np.ascontiguousarray(h0t),
            "c0": np.ascontiguousarray(
                c1[:, s * HS:(s + 1) * HS].astype(np.float32)),
            "bps": np.ascontiguousarray(bp[rows][None, :].astype(np.float16)),
            "bout": np.ascontiguousarray(
                b_out[s * 64:(s + 1) * 64][None, :].astype(np.float16)),
            "ones": np.ones((1, B), np.float16),
            "ident": np.eye(B, dtype=np.float32),
        })
    return in_maps


# ---------------------------------------------------------------------------
# harness entry point
# ---------------------------------------------------------------------------

def run(inputs, trace=False):
    nc = build_nc(T, debug=False)
    in_maps = prepare_inputs(inputs)
    from concourse import bass_utils
    return bass_utils.run_bass_kernel_spmd(
        nc, in_maps, core_ids=list(range(NC)), trace=trace)


def assemble_output(res):
    out = np.empty((B, T, MEL), np.float32)
    for s in range(NC):
        ys = res.results[s]["yout"]          # [T, B, 64]
        out[:, :, s * 64:(s + 1) * 64] = ys.transpose(1, 0, 2)
    return np.ascontiguousarray(out)


def kernel(**inputs):
    """Full-input/full-output entry. Distributes across 8 NeuronCores (TP over
    the 4H gate dim) internally; returns y [B, T, MEL] float32."""
    return assemble_output(run(inputs))
